# revision 3
# baseline (speedup 1.0000x reference)
"""Winding-number field (differentiable voxelizer) on 8 Trainium2 NeuronCores.

Looped variant: the measured per-run cost on this backend is ~60-90us per
STATIC instruction (program size), while dynamic instructions are nearly free
(probe: 512 matmuls in a For_i loop with an 8-instruction body cost the same
wall time as 64 unrolled matmuls). So the unrolled baseline (1352 static
instructions, ~83ms) is rebuilt as hardware loops:

  for j in For_i(n_ug):        # 8 groups of 8 vert tiles
    wslot <- vl5[:, j*1024 +: 1024]   (dynamic-offset copy: matmul weights
    nslot <- n4[:, j*32 +: 32]         must have static addresses)
    for i in For_i(n_pb):      # 8 point blocks
      8x mm1 (r2 via Gram form) -> 4x Ln -> 1x Exp -> 8x mm2 (PSUM acc)
      outsb[:, i*512 +: 512] += acc    (VectorE, dynamic offset)

kernel4 layout (one For_i over t in [0, reps*8), i = t % 8 the point block):
  - vert tiles fully unrolled in the body: mm1/mm2 weights (vl5 / n4 tiles)
    are static SBUF slices, so no per-iteration weight staging and no
    dynamic-offset register ops on the PE stream. Only the point-block copy
    (pslot <- pts5[:, i*512 +: 512]) and the final result copy use dynamic
    offsets, both on VectorE.
  - each iteration computes the FULL contraction for one point block: each
    vert group's mm2 closes its own PSUM accumulation group (ping-pong acc
    banks), and groups combine in SBUF (copy for group 0, add for the rest),
    overwriting outsb[:, i*512 +: 512] — iterations are idempotent, so the
    reps used for marginal timing are a pure loop-bound change (identical
    static program, identical output).
  - the PE stream is software-pipelined: mm2 of vert group g is emitted
    after mm1 of group g+1, so the PE never waits on the Exp activation
    except at the first group; s is double-buffered for this.
  - mm2 in bf16 (s and n4): PE moving rate is 1 cycle/row vs fp32's 4.
    The host near-pair correction replicates the bf16 round-to-nearest-even
    via ml_dtypes, so the correction stays exact; far-pair bf16 noise is
    ~0.5% per term, incoherent, and far terms are small vs the field norm.

Strategy otherwise identical to the unrolled baseline (see git history):
data-parallel over query points, host computes areaic normals bit-exactly
and corrects all pairs with r < RCUT in fp64.
"""

import os
import sys

import numpy as np

for _p in ("/opt/trn_rl_repo", "/root/.axon_site/_ro/trn_rl_repo"):
    if _p not in sys.path and os.path.isdir(_p):
        sys.path.insert(0, _p)

from contextlib import ExitStack

import concourse.bass as bass  # noqa: E402
import concourse.tile as tile  # noqa: E402
from concourse import bacc, mybir  # noqa: E402
from concourse.bass import ds  # noqa: E402
from concourse.bass_utils import run_bass_kernel_spmd  # noqa: E402

EPS = 1e-8          # reference epsilon in 1/(r^3 + EPS)
B_REG = 1e-4        # device regularizer: s = (r2 + B_REG)^-1.5
RCUT = 0.3          # host-corrected pair radius
FOUR_PI = 4.0 * np.pi

N_CORES = 8
V = 8192
P = 32768
PC = P // N_CORES         # 4096 points per core
PB = 512                  # point block (one fp32 matmul moving limit / PSUM bank)
VT = 128                  # vert tile (partition dim)
VG = 2                    # vert tiles per Ln chunk (FD = VG*PB = 1024)
UG = 8                    # vert tiles per loop iteration (FD_ug = UG*PB = 4096)
F32 = mybir.dt.float32
BF16 = mybir.dt.bfloat16

_NC_CACHE = {}


class _OneSetBacc(bacc.Bacc):
    """Bacc whose activation-table pass only sees `natural_log_exp_and_others`
    (contains ln, exp, copy) so a single ACT_TABLE_LOAD is hoisted instead of
    one per Ln<->Exp alternation."""

    def insert_act_table_loads(self):
        import bass_rust as _bass_rust
        from concourse.hw_specs import get_activation_tables

        has_activation = any(
            isinstance(i, mybir.InstActivation)
            for b in self.main_func.blocks
            for i in b.instructions
        )
        if not has_activation:
            return
        keep = {"natural_log_exp_and_others"}
        tables = [(k, v if k in keep else set())
                  for k, v in get_activation_tables(self.m.arch).items()]
        assert any(v for _, v in tables), "required activation sets missing"
        _bass_rust.insert_act_table_loads(self, tables)


def _build_nc(reps=1):
    """Build the SPMD Bass module (same program for all 8 cores).

    reps>1 repeats the whole computation (identical output) — used only for
    marginal-time measurement: device_time ~= (wall(N) - wall(1)) / (N-1)."""
    nc = _OneSetBacc("TRN2", target_bir_lowering=False, debug=False)

    vl13_d = nc.declare_dram_parameter("vl13", [13, V], BF16, isOutput=False)
    pts13_d = nc.declare_dram_parameter("pts13", [13, PC], BF16, isOutput=False)
    n4_d = nc.declare_dram_parameter("n4", [VT, (V // VT) * 4], F32, isOutput=False)
    out4_d = nc.declare_dram_parameter("out4", [4, PC], F32, isOutput=True)

    n_pb = PC // PB                 # 8 point blocks
    n_vt = V // VT                  # 64 vert tiles
    n_ug = n_vt // UG               # 8 vert-tile groups (unrolled in body)
    n_ch = UG // VG                 # 4 Ln chunks per group

    with ExitStack() as ctx:
        tc = ctx.enter_context(tile.TileContext(nc))
        consts = ctx.enter_context(tc.tile_pool(name="consts", bufs=1))
        psum_r2 = ctx.enter_context(tc.tile_pool(name="psum_r2", bufs=3, space="PSUM"))
        psum_out = ctx.enter_context(tc.tile_pool(name="psum_out", bufs=2, space="PSUM"))

        vl13 = consts.tile([13, V], BF16)
        pts13 = consts.tile([13, PC], BF16)
        n4 = consts.tile([VT, (V // VT) * 4], F32)
        n4b = consts.tile([VT, (V // VT) * 4], BF16)
        outsb = consts.tile([4, PC], F32)
        sqrt_bias = consts.tile([VT, 1], F32)
        pslot = consts.tile([13, PB], BF16)
        u = consts.tile([VT, UG * PB], F32)
        s_bufs = [consts.tile([VT, UG * PB], BF16, name=f"sbuf{b}")
                  for b in range(2)]
        nc.vector.memset(sqrt_bias[:], B_REG)
        nc.sync.dma_start(out=vl13[:], in_=vl13_d.ap())
        nc.sync.dma_start(out=pts13[:], in_=pts13_d.ap())
        nc.sync.dma_start(out=n4[:], in_=n4_d.ap())
        nc.vector.tensor_copy(n4b[:], n4[:])

        def mm1_group(ug, i):
            """r2 for vert group ug -> Ln into u -> Exp into s[ug%2]."""
            for ch in range(n_ch):
                r2 = psum_r2.tile([VT, VG * PB], F32, tag="r2")
                for t in range(VG):
                    vt = ug * UG + ch * VG + t
                    nc.tensor.matmul(
                        r2[:, t * PB:(t + 1) * PB],
                        vl13[:, vt * VT:(vt + 1) * VT],
                        pslot[:],
                        start=True,
                        stop=True,
                    )
                # u = Ln(r2 + B_REG)   (s = Exp(-1.5*u) = (r2+b)^-1.5)
                nc.scalar.activation(u[:, ch * VG * PB:(ch + 1) * VG * PB],
                                     r2[:],
                                     mybir.ActivationFunctionType.Ln,
                                     bias=sqrt_bias[:])
            nc.scalar.activation(s_bufs[ug % 2][:], u[:],
                                 mybir.ActivationFunctionType.Exp,
                                 scale=-1.5)

        def mm2_group(ug, osl):
            s = s_bufs[ug % 2]
            acc = psum_out.tile([4, PB], F32, tag="acc")
            for k in range(UG):
                vt = ug * UG + k
                nc.tensor.matmul(
                    acc[:],
                    n4b[:, vt * 4:(vt + 1) * 4],
                    s[:, k * PB:(k + 1) * PB],
                    start=(k == 0),
                    stop=(k == UG - 1),
                )
            if ug == 0:
                nc.vector.tensor_copy(osl, acc[:])
            else:
                nc.vector.tensor_add(osl, osl, acc[:])

        with tc.For_i(0, reps * n_pb) as t_iv:
            i = t_iv % n_pb        # point block (reps wrap around, idempotent)
            nc.vector.tensor_copy(pslot[:], pts13[:, ds(i * PB, PB)])
            osl = outsb[:, ds(i * PB, PB)]
            # software-pipelined PE stream: mm2 of group ug issues after mm1
            # of group ug+1, so the PE only waits for Exp at the first group
            mm1_group(0, i)
            for ug in range(1, n_ug):
                mm1_group(ug, i)
                mm2_group(ug - 1, osl)
            mm2_group(n_ug - 1, osl)
        nc.sync.dma_start(out=out4_d.ap(), in_=outsb[:])
    nc.finalize()
    return nc


# ------------------------- host-side numerics --------------------------------
def _preprocess_mesh(verts, faces):
    """Bit-exact replica of the reference's areaic normals: jax fp32 on CPU."""
    import jax
    import jax.numpy as jnp

    with jax.default_device(jax.devices("cpu")[0]):
        v = jnp.asarray(verts, jnp.float32)
        f = jnp.asarray(np.asarray(faces).astype(np.int32))
        fv = v[f]
        A = fv[:, 1] - fv[:, 0]
        Bv = fv[:, 2] - fv[:, 1]
        C = fv[:, 0] - fv[:, 2]

        def corner_angle(u, w):
            c = -jnp.sum(u * w, axis=1) / (
                EPS + jnp.linalg.norm(u, axis=1) * jnp.linalg.norm(w, axis=1))
            return jnp.arccos(jnp.clip(c, -1.0, 1.0))

        angles = jnp.stack(
            [corner_angle(C, A), corner_angle(A, Bv), corner_angle(Bv, C)], axis=1)
        s2 = jnp.sin(2.0 * angles)
        w = s2 / (jnp.sum(s2, axis=-1, keepdims=True) + EPS)
        w = (w[:, [2, 0, 1]] + w[:, [1, 2, 0]]) / 2.0

        fn = jnp.cross(A, Bv)
        areas = 0.5 * jnp.linalg.norm(fn, axis=1)

        nv = v.shape[0]
        idx = f.reshape(-1)
        dual_v = jax.ops.segment_sum((w * areas[:, None]).reshape(-1), idx,
                                     num_segments=nv)
        vn = jax.ops.segment_sum(jnp.repeat(fn, 3, axis=0), idx, num_segments=nv)
        vn = vn / (jnp.linalg.norm(vn, axis=1, keepdims=True) + EPS)
        na = np.asarray(vn * dual_v[:, None])
    d = np.sum(na.astype(np.float64) * np.asarray(verts, np.float64), axis=1)
    return na, d.astype(np.float32)


def _near_pairs(points, verts, rcut):
    """(point, vert) pairs with |p-v| < rcut via grid hashing (pure numpy)."""
    from collections import defaultdict

    pts = points.astype(np.float64)
    vts = verts.astype(np.float64)
    vcell = np.floor(vts / rcut).astype(np.int64)
    vmap = defaultdict(list)
    for j, c in enumerate(map(tuple, vcell)):
        vmap[c].append(j)
    vmap = {k: np.asarray(vs) for k, vs in vmap.items()}
    pcell = np.floor(pts / rcut).astype(np.int64)
    order = np.lexsort((pcell[:, 2], pcell[:, 1], pcell[:, 0]))
    pc_sorted = pcell[order]
    bounds = np.nonzero(np.any(np.diff(pc_sorted, axis=0) != 0, axis=1))[0] + 1
    starts = np.concatenate([[0], bounds])
    ends = np.concatenate([bounds, [len(order)]])
    ip_list, iv_list = [], []
    for s0, e0 in zip(starts, ends):
        pidx = order[s0:e0]
        c = pc_sorted[s0]
        cand = [vmap[k] for k in
                ((c[0] + dx, c[1] + dy, c[2] + dz)
                 for dx in (-1, 0, 1) for dy in (-1, 0, 1) for dz in (-1, 0, 1))
                if k in vmap]
        if not cand:
            continue
        cand = np.concatenate(cand)
        diff = vts[None, cand, :] - pts[pidx, None, :]
        r2 = np.sum(diff * diff, axis=2)
        ii, jj = np.nonzero(r2 < rcut * rcut)
        ip_list.append(pidx[ii])
        iv_list.append(cand[jj])
    if not ip_list:
        return np.zeros(0, np.int64), np.zeros(0, np.int64)
    return np.concatenate(ip_list), np.concatenate(iv_list)


def _split_bf16(x32):
    """x32 (fp32) -> (hi, lo) bf16 pair with hi + lo ~= x32."""
    import ml_dtypes

    bf = ml_dtypes.bfloat16
    hi = x32.astype(bf)
    lo = (x32 - hi.astype(np.float32)).astype(bf)
    return hi, lo


def _host_correction(points32, verts32, na, d32, pp32, vv32):
    """wf_corr[p] = sum_near [true_term - device_term_pred] / 4pi.

    The device computes r2 from 13 bf16 hi/lo channel products (exact
    products, fp32 PSUM accumulation), s = Exp(-1.5*Ln(r2+B_REG)) stored in
    bf16, and contracts against bf16 (na|d) weights. All bf16 roundings are
    replicated here exactly (round-to-nearest-even via ml_dtypes); only the
    device's fp32 accumulation order (~1e-6 abs on r2) and the fp32
    activation rounding are approximated by fp64."""
    import ml_dtypes

    bf = ml_dtypes.bfloat16
    ip, iv = _near_pairs(points32, verts32, RCUT)
    p = points32.astype(np.float64)[ip]
    v = verts32.astype(np.float64)[iv]
    diff = v - p
    r2t = np.sum(diff * diff, axis=1)
    s_true = 1.0 / (r2t ** 1.5 + EPS)
    g_true = np.sum(na.astype(np.float64)[iv] * diff, axis=1)
    # replicate the device's 13-channel split-bf16 Gram r2
    vhi, vlo = _split_bf16(verts32)
    q32 = (np.float32(-2.0) * points32).astype(np.float32)
    qhi, qlo = _split_bf16(q32)
    vvhi, vvlo = _split_bf16(vv32)
    pphi, pplo = _split_bf16(pp32)
    vh = vhi.astype(np.float64)[iv]
    vl_ = vlo.astype(np.float64)[iv]
    qh = qhi.astype(np.float64)[ip]
    ql = qlo.astype(np.float64)[ip]
    r2d = (np.sum(vh * qh + vh * ql + vl_ * qh, axis=1)
           + vvhi.astype(np.float64)[iv] + vvlo.astype(np.float64)[iv]
           + pphi.astype(np.float64)[ip] + pplo.astype(np.float64)[ip])
    r2d = np.maximum(r2d, 0.0)
    s_dev = ((r2d + B_REG) ** -1.5).astype(np.float32).astype(bf).astype(np.float64)
    na_bf = na.astype(np.float32).astype(bf).astype(np.float64)
    d_bf = d32.astype(bf).astype(np.float64)
    g_dev = d_bf[iv] - np.sum(p * na_bf[iv], axis=1)
    corr = (s_true * g_true - s_dev * g_dev) / FOUR_PI
    return np.bincount(ip, weights=corr, minlength=points32.shape[0])


# ------------------------------- entry point ---------------------------------
def _prepare(verts, points, faces):
    verts32 = np.ascontiguousarray(np.asarray(verts, np.float32))
    points32 = np.ascontiguousarray(np.asarray(points, np.float32))

    na, d = _preprocess_mesh(verts32, faces)

    import ml_dtypes

    bf = ml_dtypes.bfloat16
    vv32 = np.sum(verts32.astype(np.float64) ** 2, axis=1).astype(np.float32)
    pp32 = np.sum(points32.astype(np.float64) ** 2, axis=1).astype(np.float32)

    vhi, vlo = _split_bf16(verts32)
    q32 = (np.float32(-2.0) * points32).astype(np.float32)
    qhi, qlo = _split_bf16(q32)
    vvhi, vvlo = _split_bf16(vv32)
    pphi, pplo = _split_bf16(pp32)

    vl13 = np.zeros((13, V), bf)
    vl13[0:3] = vhi.T
    vl13[3:6] = vhi.T
    vl13[6:9] = vlo.T
    vl13[9] = vvhi
    vl13[10] = vvlo
    vl13[11] = 1.0
    vl13[12] = 1.0

    # n4: lhsT tiles for mm2 — n4[vp, vt*4+j] = [na | d][vt*128+vp, j]
    nmat = np.concatenate([na.astype(np.float32), d[:, None]], axis=1)  # (V,4)
    n4 = np.ascontiguousarray(
        nmat.reshape(V // VT, VT, 4).transpose(1, 0, 2).reshape(VT, (V // VT) * 4))

    in_maps = []
    for c in range(N_CORES):
        sl = slice(c * PC, (c + 1) * PC)
        pts13 = np.zeros((13, PC), bf)
        pts13[0:3] = qhi[sl].T
        pts13[3:6] = qlo[sl].T
        pts13[6:9] = qhi[sl].T
        pts13[9] = 1.0
        pts13[10] = 1.0
        pts13[11] = pphi[sl]
        pts13[12] = pplo[sl]
        in_maps.append({"vl13": vl13, "pts13": pts13, "n4": n4})
    return in_maps, verts32, points32, (na, d), pp32, vv32


def _finish(core_outs, verts32, points32, nad, pp32, vv32):
    """core_outs: list of (4, PC) arrays. Combine + near-pair correction."""
    wf = np.empty(P, np.float64)
    for c in range(N_CORES):
        sl = slice(c * PC, (c + 1) * PC)
        o = np.asarray(core_outs[c], np.float64)
        pd = points32[sl].astype(np.float64)
        wf[sl] = (o[3] - pd[:, 0] * o[0] - pd[:, 1] * o[1] - pd[:, 2] * o[2]) / FOUR_PI
    na, d32 = nad
    wf += _host_correction(points32, verts32, na, d32, pp32, vv32)
    return wf.astype(np.float32)


def kernel(verts, points, faces):
    import time

    in_maps, verts32, points32, na, pp32, vv32 = _prepare(verts, points, faces)
    last_err = None
    for attempt in range(3):
        try:
            if "nc" not in _NC_CACHE:
                _NC_CACHE["nc"] = _build_nc()
            res = run_bass_kernel_spmd(_NC_CACHE["nc"], in_maps,
                                       list(range(N_CORES)))
            core_outs = [np.asarray(res.results[c]["out4"])
                         for c in range(N_CORES)]
            break
        except Exception as e:  # transient axon/NRT faults: rebuild + retry
            last_err = e
            _NC_CACHE.clear()
            time.sleep(5 * (attempt + 1))
    else:
        raise last_err
    return _finish(core_outs, verts32, points32, na, pp32, vv32)


# revision 4
# speedup vs baseline: 1.1713x; 1.1713x over previous
"""Winding-number field (differentiable voxelizer) on 8 Trainium2 NeuronCores.

Looped variant: the measured per-run cost on this backend is ~60-90us per
STATIC instruction (program size), while dynamic instructions are nearly free
(probe: 512 matmuls in a For_i loop with an 8-instruction body cost the same
wall time as 64 unrolled matmuls). So the unrolled baseline (1352 static
instructions, ~83ms) is rebuilt as hardware loops:

  for j in For_i(n_ug):        # 8 groups of 8 vert tiles
    wslot <- vl5[:, j*1024 +: 1024]   (dynamic-offset copy: matmul weights
    nslot <- n4[:, j*32 +: 32]         must have static addresses)
    for i in For_i(n_pb):      # 8 point blocks
      8x mm1 (r2 via Gram form) -> 4x Ln -> 1x Exp -> 8x mm2 (PSUM acc)
      outsb[:, i*512 +: 512] += acc    (VectorE, dynamic offset)

kernel4 layout (one For_i over t in [0, reps*8), i = t % 8 the point block):
  - vert tiles fully unrolled in the body: mm1/mm2 weights (vl5 / n4 tiles)
    are static SBUF slices, so no per-iteration weight staging and no
    dynamic-offset register ops on the PE stream. Only the point-block copy
    (pslot <- pts5[:, i*512 +: 512]) and the final result copy use dynamic
    offsets, both on VectorE.
  - each iteration computes the FULL contraction for one point block: each
    vert group's mm2 closes its own PSUM accumulation group (ping-pong acc
    banks), and groups combine in SBUF (copy for group 0, add for the rest),
    overwriting outsb[:, i*512 +: 512] — iterations are idempotent, so the
    reps used for marginal timing are a pure loop-bound change (identical
    static program, identical output).
  - the PE stream is software-pipelined: mm2 of vert group g is emitted
    after mm1 of group g+1, so the PE never waits on the Exp activation
    except at the first group; s is double-buffered for this.
  - mm2 in bf16 (s and n4): PE moving rate is 1 cycle/row vs fp32's 4.
    The host near-pair correction replicates the bf16 round-to-nearest-even
    via ml_dtypes, so the correction stays exact; far-pair bf16 noise is
    ~0.5% per term, incoherent, and far terms are small vs the field norm.

Strategy otherwise identical to the unrolled baseline (see git history):
data-parallel over query points, host computes areaic normals bit-exactly
and corrects all pairs with r < RCUT in fp64.
"""

import os
import sys

import numpy as np

for _p in ("/opt/trn_rl_repo", "/root/.axon_site/_ro/trn_rl_repo"):
    if _p not in sys.path and os.path.isdir(_p):
        sys.path.insert(0, _p)

from contextlib import ExitStack

import concourse.bass as bass  # noqa: E402
import concourse.tile as tile  # noqa: E402
from concourse import bacc, mybir  # noqa: E402
from concourse.bass import ds  # noqa: E402
from concourse.bass_utils import run_bass_kernel_spmd  # noqa: E402

EPS = 1e-8          # reference epsilon in 1/(r^3 + EPS)
B_REG = 1e-4        # device regularizer: s = (r2 + B_REG)^-1.5
RCUT = 0.3          # host-corrected pair radius
FOUR_PI = 4.0 * np.pi

N_CORES = 8
V = 8192
P = 32768
PC = P // N_CORES         # 4096 points per core
PB = 512                  # point block (one fp32 matmul moving limit / PSUM bank)
VT = 128                  # vert tile (partition dim)
VG = 2                    # vert tiles per Ln chunk (FD = VG*PB = 1024)
UG = 8                    # vert tiles per loop iteration (FD_ug = UG*PB = 4096)
F32 = mybir.dt.float32
BF16 = mybir.dt.bfloat16

_NC_CACHE = {}


class _OneSetBacc(bacc.Bacc):
    """Bacc whose activation-table pass only sees `natural_log_exp_and_others`
    (contains ln, exp, copy) so a single ACT_TABLE_LOAD is hoisted instead of
    one per Ln<->Exp alternation."""

    def insert_act_table_loads(self):
        import bass_rust as _bass_rust
        from concourse.hw_specs import get_activation_tables

        has_activation = any(
            isinstance(i, mybir.InstActivation)
            for b in self.main_func.blocks
            for i in b.instructions
        )
        if not has_activation:
            return
        keep = {"natural_log_exp_and_others"}
        tables = [(k, v if k in keep else set())
                  for k, v in get_activation_tables(self.m.arch).items()]
        assert any(v for _, v in tables), "required activation sets missing"
        _bass_rust.insert_act_table_loads(self, tables)


def _build_nc(reps=1):
    """Build the SPMD Bass module (same program for all 8 cores).

    reps>1 repeats the whole computation (identical output) — used only for
    marginal-time measurement: device_time ~= (wall(N) - wall(1)) / (N-1)."""
    nc = _OneSetBacc("TRN2", target_bir_lowering=False, debug=False)

    vl13_d = nc.declare_dram_parameter("vl13", [13, V], BF16, isOutput=False)
    pts13_d = nc.declare_dram_parameter("pts13", [13, PC], BF16, isOutput=False)
    n4_d = nc.declare_dram_parameter("n4", [VT, (V // VT) * 4], F32, isOutput=False)
    out4_d = nc.declare_dram_parameter("out4", [4, PC], F32, isOutput=True)

    n_pb = PC // PB                 # 8 point blocks
    n_vt = V // VT                  # 64 vert tiles
    n_ug = n_vt // UG               # 8 vert-tile groups (unrolled in body)
    n_ch = UG // VG                 # 4 Ln chunks per group

    with ExitStack() as ctx:
        tc = ctx.enter_context(tile.TileContext(nc))
        consts = ctx.enter_context(tc.tile_pool(name="consts", bufs=1))
        psum_r2 = ctx.enter_context(tc.tile_pool(name="psum_r2", bufs=3, space="PSUM"))
        psum_out = ctx.enter_context(tc.tile_pool(name="psum_out", bufs=2, space="PSUM"))

        vl13 = consts.tile([13, V], BF16)
        pts13 = consts.tile([13, PC], BF16)
        n4 = consts.tile([VT, (V // VT) * 4], F32)
        n4b = consts.tile([VT, (V // VT) * 4], BF16)
        outsb = consts.tile([4, PC], F32)
        sqrt_bias = consts.tile([VT, 1], F32)
        pslot = consts.tile([13, 2 * PB], BF16)
        u = consts.tile([VT, UG * PB], F32)
        s_bufs = [consts.tile([VT, UG * PB], BF16, name=f"sbuf{b}")
                  for b in range(2)]
        nc.vector.memset(sqrt_bias[:], B_REG)
        nc.sync.dma_start(out=vl13[:], in_=vl13_d.ap())
        nc.sync.dma_start(out=pts13[:], in_=pts13_d.ap())
        nc.sync.dma_start(out=n4[:], in_=n4_d.ap())
        nc.vector.tensor_copy(n4b[:], n4[:])

        def mm1_group(vg):
            """r2 for (vert group, pb half) -> Ln into u -> Exp into s."""
            ug, b = vg
            for ch in range(n_ch):
                r2 = psum_r2.tile([VT, VG * PB], F32, tag="r2")
                for t in range(VG):
                    vt = ug * UG + ch * VG + t
                    nc.tensor.matmul(
                        r2[:, t * PB:(t + 1) * PB],
                        vl13[:, vt * VT:(vt + 1) * VT],
                        pslot[:, b * PB:(b + 1) * PB],
                        start=True,
                        stop=True,
                    )
                # u = Ln(r2 + B_REG)   (s = Exp(-1.5*u) = (r2+b)^-1.5)
                nc.scalar.activation(u[:, ch * VG * PB:(ch + 1) * VG * PB],
                                     r2[:],
                                     mybir.ActivationFunctionType.Ln,
                                     bias=sqrt_bias[:])
            nc.scalar.activation(s_bufs[(2 * ug + b) % 2][:], u[:],
                                 mybir.ActivationFunctionType.Exp,
                                 scale=-1.5)

        def mm2_group(vg):
            ug, b = vg
            s = s_bufs[(2 * ug + b) % 2]
            acc = psum_out.tile([4, PB], F32, tag="acc")
            for k in range(UG):
                vt = ug * UG + k
                nc.tensor.matmul(
                    acc[:],
                    n4b[:, vt * 4:(vt + 1) * 4],
                    s[:, k * PB:(k + 1) * PB],
                    start=(k == 0),
                    stop=(k == UG - 1),
                )
            if ug == 0:
                nc.vector.tensor_copy(osls[b], acc[:])
            else:
                nc.vector.tensor_add(osls[b], osls[b], acc[:])

        n_half = n_pb // 2      # two point blocks per iteration
        with tc.For_i(0, reps * n_half) as t_iv:
            # pair of point blocks (reps wrap around, idempotent)
            i2 = (t_iv % n_half) * 2
            nc.vector.tensor_copy(pslot[:], pts13[:, ds(i2 * PB, 2 * PB)])
            osls = [outsb[:, ds((i2 + b) * PB, PB)] for b in range(2)]
            # 16 virtual groups (vert group, point-block half); mm2 of group
            # g issues after mm1 of group g+1, so the PE only waits for Exp
            # at the first group
            vgs = [(ug, b) for ug in range(n_ug) for b in range(2)]
            mm1_group(vgs[0])
            for g in range(1, len(vgs)):
                mm1_group(vgs[g])
                mm2_group(vgs[g - 1])
            mm2_group(vgs[-1])
        nc.sync.dma_start(out=out4_d.ap(), in_=outsb[:])
    nc.finalize()
    return nc


# ------------------------- host-side numerics --------------------------------
def _preprocess_mesh(verts, faces):
    """Bit-exact replica of the reference's areaic normals: jax fp32 on CPU."""
    import jax
    import jax.numpy as jnp

    with jax.default_device(jax.devices("cpu")[0]):
        v = jnp.asarray(verts, jnp.float32)
        f = jnp.asarray(np.asarray(faces).astype(np.int32))
        fv = v[f]
        A = fv[:, 1] - fv[:, 0]
        Bv = fv[:, 2] - fv[:, 1]
        C = fv[:, 0] - fv[:, 2]

        def corner_angle(u, w):
            c = -jnp.sum(u * w, axis=1) / (
                EPS + jnp.linalg.norm(u, axis=1) * jnp.linalg.norm(w, axis=1))
            return jnp.arccos(jnp.clip(c, -1.0, 1.0))

        angles = jnp.stack(
            [corner_angle(C, A), corner_angle(A, Bv), corner_angle(Bv, C)], axis=1)
        s2 = jnp.sin(2.0 * angles)
        w = s2 / (jnp.sum(s2, axis=-1, keepdims=True) + EPS)
        w = (w[:, [2, 0, 1]] + w[:, [1, 2, 0]]) / 2.0

        fn = jnp.cross(A, Bv)
        areas = 0.5 * jnp.linalg.norm(fn, axis=1)

        nv = v.shape[0]
        idx = f.reshape(-1)
        dual_v = jax.ops.segment_sum((w * areas[:, None]).reshape(-1), idx,
                                     num_segments=nv)
        vn = jax.ops.segment_sum(jnp.repeat(fn, 3, axis=0), idx, num_segments=nv)
        vn = vn / (jnp.linalg.norm(vn, axis=1, keepdims=True) + EPS)
        na = np.asarray(vn * dual_v[:, None])
    d = np.sum(na.astype(np.float64) * np.asarray(verts, np.float64), axis=1)
    return na, d.astype(np.float32)


def _near_pairs(points, verts, rcut):
    """(point, vert) pairs with |p-v| < rcut via grid hashing (pure numpy)."""
    from collections import defaultdict

    pts = points.astype(np.float64)
    vts = verts.astype(np.float64)
    vcell = np.floor(vts / rcut).astype(np.int64)
    vmap = defaultdict(list)
    for j, c in enumerate(map(tuple, vcell)):
        vmap[c].append(j)
    vmap = {k: np.asarray(vs) for k, vs in vmap.items()}
    pcell = np.floor(pts / rcut).astype(np.int64)
    order = np.lexsort((pcell[:, 2], pcell[:, 1], pcell[:, 0]))
    pc_sorted = pcell[order]
    bounds = np.nonzero(np.any(np.diff(pc_sorted, axis=0) != 0, axis=1))[0] + 1
    starts = np.concatenate([[0], bounds])
    ends = np.concatenate([bounds, [len(order)]])
    ip_list, iv_list = [], []
    for s0, e0 in zip(starts, ends):
        pidx = order[s0:e0]
        c = pc_sorted[s0]
        cand = [vmap[k] for k in
                ((c[0] + dx, c[1] + dy, c[2] + dz)
                 for dx in (-1, 0, 1) for dy in (-1, 0, 1) for dz in (-1, 0, 1))
                if k in vmap]
        if not cand:
            continue
        cand = np.concatenate(cand)
        diff = vts[None, cand, :] - pts[pidx, None, :]
        r2 = np.sum(diff * diff, axis=2)
        ii, jj = np.nonzero(r2 < rcut * rcut)
        ip_list.append(pidx[ii])
        iv_list.append(cand[jj])
    if not ip_list:
        return np.zeros(0, np.int64), np.zeros(0, np.int64)
    return np.concatenate(ip_list), np.concatenate(iv_list)


def _split_bf16(x32):
    """x32 (fp32) -> (hi, lo) bf16 pair with hi + lo ~= x32."""
    import ml_dtypes

    bf = ml_dtypes.bfloat16
    hi = x32.astype(bf)
    lo = (x32 - hi.astype(np.float32)).astype(bf)
    return hi, lo


def _host_correction(points32, verts32, na, d32, pp32, vv32):
    """wf_corr[p] = sum_near [true_term - device_term_pred] / 4pi.

    The device computes r2 from 13 bf16 hi/lo channel products (exact
    products, fp32 PSUM accumulation), s = Exp(-1.5*Ln(r2+B_REG)) stored in
    bf16, and contracts against bf16 (na|d) weights. All bf16 roundings are
    replicated here exactly (round-to-nearest-even via ml_dtypes); only the
    device's fp32 accumulation order (~1e-6 abs on r2) and the fp32
    activation rounding are approximated by fp64."""
    import ml_dtypes

    bf = ml_dtypes.bfloat16
    ip, iv = _near_pairs(points32, verts32, RCUT)
    p = points32.astype(np.float64)[ip]
    v = verts32.astype(np.float64)[iv]
    diff = v - p
    r2t = np.sum(diff * diff, axis=1)
    s_true = 1.0 / (r2t ** 1.5 + EPS)
    g_true = np.sum(na.astype(np.float64)[iv] * diff, axis=1)
    # replicate the device's 13-channel split-bf16 Gram r2
    vhi, vlo = _split_bf16(verts32)
    q32 = (np.float32(-2.0) * points32).astype(np.float32)
    qhi, qlo = _split_bf16(q32)
    vvhi, vvlo = _split_bf16(vv32)
    pphi, pplo = _split_bf16(pp32)
    vh = vhi.astype(np.float64)[iv]
    vl_ = vlo.astype(np.float64)[iv]
    qh = qhi.astype(np.float64)[ip]
    ql = qlo.astype(np.float64)[ip]
    r2d = (np.sum(vh * qh + vh * ql + vl_ * qh, axis=1)
           + vvhi.astype(np.float64)[iv] + vvlo.astype(np.float64)[iv]
           + pphi.astype(np.float64)[ip] + pplo.astype(np.float64)[ip])
    r2d = np.maximum(r2d, 0.0)
    s_dev = ((r2d + B_REG) ** -1.5).astype(np.float32).astype(bf).astype(np.float64)
    na_bf = na.astype(np.float32).astype(bf).astype(np.float64)
    d_bf = d32.astype(bf).astype(np.float64)
    g_dev = d_bf[iv] - np.sum(p * na_bf[iv], axis=1)
    corr = (s_true * g_true - s_dev * g_dev) / FOUR_PI
    return np.bincount(ip, weights=corr, minlength=points32.shape[0])


# ------------------------------- entry point ---------------------------------
def _prepare(verts, points, faces):
    verts32 = np.ascontiguousarray(np.asarray(verts, np.float32))
    points32 = np.ascontiguousarray(np.asarray(points, np.float32))

    na, d = _preprocess_mesh(verts32, faces)

    import ml_dtypes

    bf = ml_dtypes.bfloat16
    vv32 = np.sum(verts32.astype(np.float64) ** 2, axis=1).astype(np.float32)
    pp32 = np.sum(points32.astype(np.float64) ** 2, axis=1).astype(np.float32)

    vhi, vlo = _split_bf16(verts32)
    q32 = (np.float32(-2.0) * points32).astype(np.float32)
    qhi, qlo = _split_bf16(q32)
    vvhi, vvlo = _split_bf16(vv32)
    pphi, pplo = _split_bf16(pp32)

    vl13 = np.zeros((13, V), bf)
    vl13[0:3] = vhi.T
    vl13[3:6] = vhi.T
    vl13[6:9] = vlo.T
    vl13[9] = vvhi
    vl13[10] = vvlo
    vl13[11] = 1.0
    vl13[12] = 1.0

    # n4: lhsT tiles for mm2 — n4[vp, vt*4+j] = [na | d][vt*128+vp, j]
    nmat = np.concatenate([na.astype(np.float32), d[:, None]], axis=1)  # (V,4)
    n4 = np.ascontiguousarray(
        nmat.reshape(V // VT, VT, 4).transpose(1, 0, 2).reshape(VT, (V // VT) * 4))

    in_maps = []
    for c in range(N_CORES):
        sl = slice(c * PC, (c + 1) * PC)
        pts13 = np.zeros((13, PC), bf)
        pts13[0:3] = qhi[sl].T
        pts13[3:6] = qlo[sl].T
        pts13[6:9] = qhi[sl].T
        pts13[9] = 1.0
        pts13[10] = 1.0
        pts13[11] = pphi[sl]
        pts13[12] = pplo[sl]
        in_maps.append({"vl13": vl13, "pts13": pts13, "n4": n4})
    return in_maps, verts32, points32, (na, d), pp32, vv32


def _finish(core_outs, verts32, points32, nad, pp32, vv32):
    """core_outs: list of (4, PC) arrays. Combine + near-pair correction."""
    wf = np.empty(P, np.float64)
    for c in range(N_CORES):
        sl = slice(c * PC, (c + 1) * PC)
        o = np.asarray(core_outs[c], np.float64)
        pd = points32[sl].astype(np.float64)
        wf[sl] = (o[3] - pd[:, 0] * o[0] - pd[:, 1] * o[1] - pd[:, 2] * o[2]) / FOUR_PI
    na, d32 = nad
    wf += _host_correction(points32, verts32, na, d32, pp32, vv32)
    return wf.astype(np.float32)


def kernel(verts, points, faces):
    import time

    in_maps, verts32, points32, na, pp32, vv32 = _prepare(verts, points, faces)
    last_err = None
    for attempt in range(3):
        try:
            if "nc" not in _NC_CACHE:
                _NC_CACHE["nc"] = _build_nc()
            res = run_bass_kernel_spmd(_NC_CACHE["nc"], in_maps,
                                       list(range(N_CORES)))
            core_outs = [np.asarray(res.results[c]["out4"])
                         for c in range(N_CORES)]
            break
        except Exception as e:  # transient axon/NRT faults: rebuild + retry
            last_err = e
            _NC_CACHE.clear()
            time.sleep(5 * (attempt + 1))
    else:
        raise last_err
    return _finish(core_outs, verts32, points32, na, pp32, vv32)


# revision 5
# speedup vs baseline: 1.2145x; 1.0369x over previous
"""Winding-number field (differentiable voxelizer) on 8 Trainium2 NeuronCores.

Looped variant: the measured per-run cost on this backend is ~60-90us per
STATIC instruction (program size), while dynamic instructions are nearly free
(probe: 512 matmuls in a For_i loop with an 8-instruction body cost the same
wall time as 64 unrolled matmuls). So the unrolled baseline (1352 static
instructions, ~83ms) is rebuilt as hardware loops:

  for j in For_i(n_ug):        # 8 groups of 8 vert tiles
    wslot <- vl5[:, j*1024 +: 1024]   (dynamic-offset copy: matmul weights
    nslot <- n4[:, j*32 +: 32]         must have static addresses)
    for i in For_i(n_pb):      # 8 point blocks
      8x mm1 (r2 via Gram form) -> 4x Ln -> 1x Exp -> 8x mm2 (PSUM acc)
      outsb[:, i*512 +: 512] += acc    (VectorE, dynamic offset)

kernel4 layout (one For_i over t in [0, reps*8), i = t % 8 the point block):
  - vert tiles fully unrolled in the body: mm1/mm2 weights (vl5 / n4 tiles)
    are static SBUF slices, so no per-iteration weight staging and no
    dynamic-offset register ops on the PE stream. Only the point-block copy
    (pslot <- pts5[:, i*512 +: 512]) and the final result copy use dynamic
    offsets, both on VectorE.
  - each iteration computes the FULL contraction for one point block: each
    vert group's mm2 closes its own PSUM accumulation group (ping-pong acc
    banks), and groups combine in SBUF (copy for group 0, add for the rest),
    overwriting outsb[:, i*512 +: 512] — iterations are idempotent, so the
    reps used for marginal timing are a pure loop-bound change (identical
    static program, identical output).
  - the PE stream is software-pipelined: mm2 of vert group g is emitted
    after mm1 of group g+1, so the PE never waits on the Exp activation
    except at the first group; s is double-buffered for this.
  - mm2 in bf16 (s and n4): PE moving rate is 1 cycle/row vs fp32's 4.
    The host near-pair correction replicates the bf16 round-to-nearest-even
    via ml_dtypes, so the correction stays exact; far-pair bf16 noise is
    ~0.5% per term, incoherent, and far terms are small vs the field norm.

Strategy otherwise identical to the unrolled baseline (see git history):
data-parallel over query points, host computes areaic normals bit-exactly
and corrects all pairs with r < RCUT in fp64.
"""

import os
import sys

import numpy as np

for _p in ("/opt/trn_rl_repo", "/root/.axon_site/_ro/trn_rl_repo"):
    if _p not in sys.path and os.path.isdir(_p):
        sys.path.insert(0, _p)

from contextlib import ExitStack

import concourse.bass as bass  # noqa: E402
import concourse.tile as tile  # noqa: E402
from concourse import bacc, mybir  # noqa: E402
from concourse.bass import ds  # noqa: E402
from concourse.bass_utils import run_bass_kernel_spmd  # noqa: E402

EPS = 1e-8          # reference epsilon in 1/(r^3 + EPS)
B_REG = 1e-4        # device regularizer: s = (r2 + B_REG)^-1.5
RCUT = 0.3          # host-corrected pair radius
FOUR_PI = 4.0 * np.pi

N_CORES = 8
V = 8192
P = 32768
PC = P // N_CORES         # 4096 points per core
PB = 512                  # point block (one fp32 matmul moving limit / PSUM bank)
VT = 128                  # vert tile (partition dim)
VG = 2                    # vert tiles per Ln chunk (FD = VG*PB = 1024)
UG = 8                    # vert tiles per loop iteration (FD_ug = UG*PB = 4096)
F32 = mybir.dt.float32
BF16 = mybir.dt.bfloat16

_NC_CACHE = {}


class _OneSetBacc(bacc.Bacc):
    """Bacc whose activation-table pass only sees
    `abs_reciprocal_sqrt_and_small` so a single ACT_TABLE_LOAD is hoisted."""

    def insert_act_table_loads(self):
        import bass_rust as _bass_rust
        from concourse.hw_specs import get_activation_tables

        has_activation = any(
            isinstance(i, mybir.InstActivation)
            for b in self.main_func.blocks
            for i in b.instructions
        )
        if not has_activation:
            return
        keep = {"abs_reciprocal_sqrt_and_small"}
        tables = [(k, v if k in keep else set())
                  for k, v in get_activation_tables(self.m.arch).items()]
        assert any(v for _, v in tables), "required activation sets missing"
        _bass_rust.insert_act_table_loads(self, tables)


def _build_nc(reps=1):
    """Build the SPMD Bass module (same program for all 8 cores).

    reps>1 repeats the whole computation (identical output) — used only for
    marginal-time measurement: device_time ~= (wall(N) - wall(1)) / (N-1)."""
    nc = _OneSetBacc("TRN2", target_bir_lowering=False, debug=False)

    vl13_d = nc.declare_dram_parameter("vl13", [13, V], BF16, isOutput=False)
    pts13_d = nc.declare_dram_parameter("pts13", [13, PC], BF16, isOutput=False)
    n4_d = nc.declare_dram_parameter("n4", [VT, (V // VT) * 4], F32, isOutput=False)
    out4_d = nc.declare_dram_parameter("out4", [4, PC], F32, isOutput=True)

    n_pb = PC // PB                 # 8 point blocks
    n_vt = V // VT                  # 64 vert tiles
    n_ug = n_vt // UG               # 8 vert-tile groups (unrolled in body)
    n_ch = UG // VG                 # 4 Ln chunks per group

    with ExitStack() as ctx:
        tc = ctx.enter_context(tile.TileContext(nc))
        consts = ctx.enter_context(tc.tile_pool(name="consts", bufs=1))
        psum_r2 = ctx.enter_context(tc.tile_pool(name="psum_r2", bufs=3, space="PSUM"))
        psum_out = ctx.enter_context(tc.tile_pool(name="psum_out", bufs=2, space="PSUM"))

        vl13 = consts.tile([13, V], BF16)
        pts13 = consts.tile([13, PC], BF16)
        n4 = consts.tile([VT, (V // VT) * 4], F32)
        n4b = consts.tile([VT, (V // VT) * 4], BF16)
        outsb = consts.tile([4, PC], F32)
        sqrt_bias = consts.tile([VT, 1], F32)
        pslot = consts.tile([13, 2 * PB], BF16)
        m_bufs = [consts.tile([VT, UG * PB], BF16, name=f"mbuf{b}")
                  for b in range(2)]
        m2 = consts.tile([VT, UG * PB], BF16)
        s_bufs = [consts.tile([VT, UG * PB], BF16, name=f"sbuf{b}")
                  for b in range(2)]
        nc.vector.memset(sqrt_bias[:], B_REG)
        nc.sync.dma_start(out=vl13[:], in_=vl13_d.ap())
        nc.sync.dma_start(out=pts13[:], in_=pts13_d.ap())
        nc.sync.dma_start(out=n4[:], in_=n4_d.ap())
        nc.vector.tensor_copy(n4b[:], n4[:])

        def mm1_group(vg):
            """r2 for (vert group, pb half) -> Ln into u -> Exp into s."""
            ug, b = vg
            for ch in range(n_ch):
                r2 = psum_r2.tile([VT, VG * PB], F32, tag="r2")
                for t in range(VG):
                    vt = ug * UG + ch * VG + t
                    nc.tensor.matmul(
                        r2[:, t * PB:(t + 1) * PB],
                        vl13[:, vt * VT:(vt + 1) * VT],
                        pslot[:, b * PB:(b + 1) * PB],
                        start=True,
                        stop=True,
                    )
                # m = (r2 + B_REG)^-0.5; s = m^3 via two bf16 DVE muls
                par = (2 * ug + b) % 2
                nc.scalar.activation(m_bufs[par][:, ch * VG * PB:
                                                 (ch + 1) * VG * PB],
                                     r2[:],
                                     mybir.ActivationFunctionType
                                     .Abs_reciprocal_sqrt,
                                     bias=sqrt_bias[:])
            nc.vector.tensor_mul(m2[:], m_bufs[par][:], m_bufs[par][:])
            nc.vector.tensor_mul(s_bufs[par][:], m2[:], m_bufs[par][:])

        def mm2_group(vg):
            ug, b = vg
            s = s_bufs[(2 * ug + b) % 2]
            acc = psum_out.tile([4, PB], F32, tag="acc")
            for k in range(UG):
                vt = ug * UG + k
                nc.tensor.matmul(
                    acc[:],
                    n4b[:, vt * 4:(vt + 1) * 4],
                    s[:, k * PB:(k + 1) * PB],
                    start=(k == 0),
                    stop=(k == UG - 1),
                )
            if ug == 0:
                nc.vector.tensor_copy(osls[b], acc[:])
            else:
                nc.vector.tensor_add(osls[b], osls[b], acc[:])

        n_half = n_pb // 2      # two point blocks per iteration
        with tc.For_i(0, reps * n_half) as t_iv:
            # pair of point blocks (reps wrap around, idempotent)
            i2 = (t_iv % n_half) * 2
            nc.vector.tensor_copy(pslot[:], pts13[:, ds(i2 * PB, 2 * PB)])
            osls = [outsb[:, ds((i2 + b) * PB, PB)] for b in range(2)]
            # 16 virtual groups (vert group, point-block half); mm2 of group
            # g issues after mm1 of group g+1, so the PE only waits for Exp
            # at the first group
            vgs = [(ug, b) for ug in range(n_ug) for b in range(2)]
            mm1_group(vgs[0])
            for g in range(1, len(vgs)):
                mm1_group(vgs[g])
                mm2_group(vgs[g - 1])
            mm2_group(vgs[-1])
        nc.sync.dma_start(out=out4_d.ap(), in_=outsb[:])
    nc.finalize()
    return nc


# ------------------------- host-side numerics --------------------------------
def _preprocess_mesh(verts, faces):
    """Bit-exact replica of the reference's areaic normals: jax fp32 on CPU."""
    import jax
    import jax.numpy as jnp

    with jax.default_device(jax.devices("cpu")[0]):
        v = jnp.asarray(verts, jnp.float32)
        f = jnp.asarray(np.asarray(faces).astype(np.int32))
        fv = v[f]
        A = fv[:, 1] - fv[:, 0]
        Bv = fv[:, 2] - fv[:, 1]
        C = fv[:, 0] - fv[:, 2]

        def corner_angle(u, w):
            c = -jnp.sum(u * w, axis=1) / (
                EPS + jnp.linalg.norm(u, axis=1) * jnp.linalg.norm(w, axis=1))
            return jnp.arccos(jnp.clip(c, -1.0, 1.0))

        angles = jnp.stack(
            [corner_angle(C, A), corner_angle(A, Bv), corner_angle(Bv, C)], axis=1)
        s2 = jnp.sin(2.0 * angles)
        w = s2 / (jnp.sum(s2, axis=-1, keepdims=True) + EPS)
        w = (w[:, [2, 0, 1]] + w[:, [1, 2, 0]]) / 2.0

        fn = jnp.cross(A, Bv)
        areas = 0.5 * jnp.linalg.norm(fn, axis=1)

        nv = v.shape[0]
        idx = f.reshape(-1)
        dual_v = jax.ops.segment_sum((w * areas[:, None]).reshape(-1), idx,
                                     num_segments=nv)
        vn = jax.ops.segment_sum(jnp.repeat(fn, 3, axis=0), idx, num_segments=nv)
        vn = vn / (jnp.linalg.norm(vn, axis=1, keepdims=True) + EPS)
        na = np.asarray(vn * dual_v[:, None])
    d = np.sum(na.astype(np.float64) * np.asarray(verts, np.float64), axis=1)
    return na, d.astype(np.float32)


def _near_pairs(points, verts, rcut):
    """(point, vert) pairs with |p-v| < rcut via grid hashing (pure numpy)."""
    from collections import defaultdict

    pts = points.astype(np.float64)
    vts = verts.astype(np.float64)
    vcell = np.floor(vts / rcut).astype(np.int64)
    vmap = defaultdict(list)
    for j, c in enumerate(map(tuple, vcell)):
        vmap[c].append(j)
    vmap = {k: np.asarray(vs) for k, vs in vmap.items()}
    pcell = np.floor(pts / rcut).astype(np.int64)
    order = np.lexsort((pcell[:, 2], pcell[:, 1], pcell[:, 0]))
    pc_sorted = pcell[order]
    bounds = np.nonzero(np.any(np.diff(pc_sorted, axis=0) != 0, axis=1))[0] + 1
    starts = np.concatenate([[0], bounds])
    ends = np.concatenate([bounds, [len(order)]])
    ip_list, iv_list = [], []
    for s0, e0 in zip(starts, ends):
        pidx = order[s0:e0]
        c = pc_sorted[s0]
        cand = [vmap[k] for k in
                ((c[0] + dx, c[1] + dy, c[2] + dz)
                 for dx in (-1, 0, 1) for dy in (-1, 0, 1) for dz in (-1, 0, 1))
                if k in vmap]
        if not cand:
            continue
        cand = np.concatenate(cand)
        diff = vts[None, cand, :] - pts[pidx, None, :]
        r2 = np.sum(diff * diff, axis=2)
        ii, jj = np.nonzero(r2 < rcut * rcut)
        ip_list.append(pidx[ii])
        iv_list.append(cand[jj])
    if not ip_list:
        return np.zeros(0, np.int64), np.zeros(0, np.int64)
    return np.concatenate(ip_list), np.concatenate(iv_list)


def _split_bf16(x32):
    """x32 (fp32) -> (hi, lo) bf16 pair with hi + lo ~= x32."""
    import ml_dtypes

    bf = ml_dtypes.bfloat16
    hi = x32.astype(bf)
    lo = (x32 - hi.astype(np.float32)).astype(bf)
    return hi, lo


def _host_correction(points32, verts32, na, d32, pp32, vv32):
    """wf_corr[p] = sum_near [true_term - device_term_pred] / 4pi.

    The device computes r2 from 13 bf16 hi/lo channel products (exact
    products, fp32 PSUM accumulation), s = cube(bf16((r2+B_REG)^-0.5))
    via the abs_reciprocal_sqrt table and bf16 VectorE multiplies, and
    contracts against bf16 (na|d) weights. All bf16 roundings are
    replicated here exactly (round-to-nearest-even via ml_dtypes); only the
    device's fp32 accumulation order (~1e-6 abs on r2) and the fp32
    activation rounding are approximated by fp64."""
    import ml_dtypes

    bf = ml_dtypes.bfloat16
    ip, iv = _near_pairs(points32, verts32, RCUT)
    p = points32.astype(np.float64)[ip]
    v = verts32.astype(np.float64)[iv]
    diff = v - p
    r2t = np.sum(diff * diff, axis=1)
    s_true = 1.0 / (r2t ** 1.5 + EPS)
    g_true = np.sum(na.astype(np.float64)[iv] * diff, axis=1)
    # replicate the device's 13-channel split-bf16 Gram r2
    vhi, vlo = _split_bf16(verts32)
    q32 = (np.float32(-2.0) * points32).astype(np.float32)
    qhi, qlo = _split_bf16(q32)
    vvhi, vvlo = _split_bf16(vv32)
    pphi, pplo = _split_bf16(pp32)
    vh = vhi.astype(np.float64)[iv]
    vl_ = vlo.astype(np.float64)[iv]
    qh = qhi.astype(np.float64)[ip]
    ql = qlo.astype(np.float64)[ip]
    r2d = (np.sum(vh * qh + vh * ql + vl_ * qh, axis=1)
           + vvhi.astype(np.float64)[iv] + vvlo.astype(np.float64)[iv]
           + pphi.astype(np.float64)[ip] + pplo.astype(np.float64)[ip])
    r2d = np.maximum(r2d, 0.0)
    m_bf = ((r2d + B_REG) ** -0.5).astype(np.float32).astype(bf).astype(np.float64)
    m2_bf = (m_bf * m_bf).astype(bf).astype(np.float64)
    s_dev = (m2_bf * m_bf).astype(bf).astype(np.float64)
    na_bf = na.astype(np.float32).astype(bf).astype(np.float64)
    d_bf = d32.astype(bf).astype(np.float64)
    g_dev = d_bf[iv] - np.sum(p * na_bf[iv], axis=1)
    corr = (s_true * g_true - s_dev * g_dev) / FOUR_PI
    return np.bincount(ip, weights=corr, minlength=points32.shape[0])


# ------------------------------- entry point ---------------------------------
def _prepare(verts, points, faces):
    verts32 = np.ascontiguousarray(np.asarray(verts, np.float32))
    points32 = np.ascontiguousarray(np.asarray(points, np.float32))

    na, d = _preprocess_mesh(verts32, faces)

    import ml_dtypes

    bf = ml_dtypes.bfloat16
    vv32 = np.sum(verts32.astype(np.float64) ** 2, axis=1).astype(np.float32)
    pp32 = np.sum(points32.astype(np.float64) ** 2, axis=1).astype(np.float32)

    vhi, vlo = _split_bf16(verts32)
    q32 = (np.float32(-2.0) * points32).astype(np.float32)
    qhi, qlo = _split_bf16(q32)
    vvhi, vvlo = _split_bf16(vv32)
    pphi, pplo = _split_bf16(pp32)

    vl13 = np.zeros((13, V), bf)
    vl13[0:3] = vhi.T
    vl13[3:6] = vhi.T
    vl13[6:9] = vlo.T
    vl13[9] = vvhi
    vl13[10] = vvlo
    vl13[11] = 1.0
    vl13[12] = 1.0

    # n4: lhsT tiles for mm2 — n4[vp, vt*4+j] = [na | d][vt*128+vp, j]
    nmat = np.concatenate([na.astype(np.float32), d[:, None]], axis=1)  # (V,4)
    n4 = np.ascontiguousarray(
        nmat.reshape(V // VT, VT, 4).transpose(1, 0, 2).reshape(VT, (V // VT) * 4))

    in_maps = []
    for c in range(N_CORES):
        sl = slice(c * PC, (c + 1) * PC)
        pts13 = np.zeros((13, PC), bf)
        pts13[0:3] = qhi[sl].T
        pts13[3:6] = qlo[sl].T
        pts13[6:9] = qhi[sl].T
        pts13[9] = 1.0
        pts13[10] = 1.0
        pts13[11] = pphi[sl]
        pts13[12] = pplo[sl]
        in_maps.append({"vl13": vl13, "pts13": pts13, "n4": n4})
    return in_maps, verts32, points32, (na, d), pp32, vv32


def _finish(core_outs, verts32, points32, nad, pp32, vv32):
    """core_outs: list of (4, PC) arrays. Combine + near-pair correction."""
    wf = np.empty(P, np.float64)
    for c in range(N_CORES):
        sl = slice(c * PC, (c + 1) * PC)
        o = np.asarray(core_outs[c], np.float64)
        pd = points32[sl].astype(np.float64)
        wf[sl] = (o[3] - pd[:, 0] * o[0] - pd[:, 1] * o[1] - pd[:, 2] * o[2]) / FOUR_PI
    na, d32 = nad
    wf += _host_correction(points32, verts32, na, d32, pp32, vv32)
    return wf.astype(np.float32)


def kernel(verts, points, faces):
    import time

    in_maps, verts32, points32, na, pp32, vv32 = _prepare(verts, points, faces)
    last_err = None
    for attempt in range(3):
        try:
            if "nc" not in _NC_CACHE:
                _NC_CACHE["nc"] = _build_nc()
            res = run_bass_kernel_spmd(_NC_CACHE["nc"], in_maps,
                                       list(range(N_CORES)))
            core_outs = [np.asarray(res.results[c]["out4"])
                         for c in range(N_CORES)]
            break
        except Exception as e:  # transient axon/NRT faults: rebuild + retry
            last_err = e
            _NC_CACHE.clear()
            time.sleep(5 * (attempt + 1))
    else:
        raise last_err
    return _finish(core_outs, verts32, points32, na, pp32, vv32)


# revision 6
# speedup vs baseline: 1.2759x; 1.0506x over previous
"""Winding-number field (differentiable voxelizer) on 8 Trainium2 NeuronCores.

Looped variant: the measured per-run cost on this backend is ~60-90us per
STATIC instruction (program size), while dynamic instructions are nearly free
(probe: 512 matmuls in a For_i loop with an 8-instruction body cost the same
wall time as 64 unrolled matmuls). So the unrolled baseline (1352 static
instructions, ~83ms) is rebuilt as hardware loops:

  for j in For_i(n_ug):        # 8 groups of 8 vert tiles
    wslot <- vl5[:, j*1024 +: 1024]   (dynamic-offset copy: matmul weights
    nslot <- n4[:, j*32 +: 32]         must have static addresses)
    for i in For_i(n_pb):      # 8 point blocks
      8x mm1 (r2 via Gram form) -> 4x Ln -> 1x Exp -> 8x mm2 (PSUM acc)
      outsb[:, i*512 +: 512] += acc    (VectorE, dynamic offset)

kernel4 layout (one For_i over t in [0, reps*8), i = t % 8 the point block):
  - vert tiles fully unrolled in the body: mm1/mm2 weights (vl5 / n4 tiles)
    are static SBUF slices, so no per-iteration weight staging and no
    dynamic-offset register ops on the PE stream. Only the point-block copy
    (pslot <- pts5[:, i*512 +: 512]) and the final result copy use dynamic
    offsets, both on VectorE.
  - each iteration computes the FULL contraction for one point block: each
    vert group's mm2 closes its own PSUM accumulation group (ping-pong acc
    banks), and groups combine in SBUF (copy for group 0, add for the rest),
    overwriting outsb[:, i*512 +: 512] — iterations are idempotent, so the
    reps used for marginal timing are a pure loop-bound change (identical
    static program, identical output).
  - the PE stream is software-pipelined: mm2 of vert group g is emitted
    after mm1 of group g+1, so the PE never waits on the Exp activation
    except at the first group; s is double-buffered for this.
  - mm2 in bf16 (s and n4): PE moving rate is 1 cycle/row vs fp32's 4.
    The host near-pair correction replicates the bf16 round-to-nearest-even
    via ml_dtypes, so the correction stays exact; far-pair bf16 noise is
    ~0.5% per term, incoherent, and far terms are small vs the field norm.

Strategy otherwise identical to the unrolled baseline (see git history):
data-parallel over query points, host computes areaic normals bit-exactly
and corrects all pairs with r < RCUT in fp64.
"""

import os
import sys

import numpy as np

for _p in ("/opt/trn_rl_repo", "/root/.axon_site/_ro/trn_rl_repo"):
    if _p not in sys.path and os.path.isdir(_p):
        sys.path.insert(0, _p)

from contextlib import ExitStack

import concourse.bass as bass  # noqa: E402
import concourse.tile as tile  # noqa: E402
from concourse import bacc, mybir  # noqa: E402
from concourse.bass import ds  # noqa: E402
from concourse.bass_utils import run_bass_kernel_spmd  # noqa: E402

EPS = 1e-8          # reference epsilon in 1/(r^3 + EPS)
B_REG = 1e-4        # device regularizer: s = (r2 + B_REG)^-1.5
RCUT = 0.3          # host-corrected pair radius
FOUR_PI = 4.0 * np.pi

N_CORES = 8
V = 8192
P = 32768
PC = P // N_CORES         # 4096 points per core
PB = 512                  # point block (one fp32 matmul moving limit / PSUM bank)
VT = 128                  # vert tile (partition dim)
VG = 2                    # vert tiles per Ln chunk (FD = VG*PB = 1024)
UG = 8                    # vert tiles per loop iteration (FD_ug = UG*PB = 4096)
F32 = mybir.dt.float32
BF16 = mybir.dt.bfloat16

_NC_CACHE = {}


class _OneSetBacc(bacc.Bacc):
    """Bacc whose activation-table pass only sees
    `abs_reciprocal_sqrt_and_small` so a single ACT_TABLE_LOAD is hoisted."""

    def insert_act_table_loads(self):
        import bass_rust as _bass_rust
        from concourse.hw_specs import get_activation_tables

        has_activation = any(
            isinstance(i, mybir.InstActivation)
            for b in self.main_func.blocks
            for i in b.instructions
        )
        if not has_activation:
            return
        keep = {"abs_reciprocal_sqrt_and_small"}
        tables = [(k, v if k in keep else set())
                  for k, v in get_activation_tables(self.m.arch).items()]
        assert any(v for _, v in tables), "required activation sets missing"
        _bass_rust.insert_act_table_loads(self, tables)


def _build_nc(reps=1):
    """Build the SPMD Bass module (same program for all 8 cores).

    reps>1 repeats the whole computation (identical output) — used only for
    marginal-time measurement: device_time ~= (wall(N) - wall(1)) / (N-1)."""
    nc = _OneSetBacc("TRN2", target_bir_lowering=False, debug=False)

    vl13_d = nc.declare_dram_parameter("vl13", [13, V], BF16, isOutput=False)
    pts13_d = nc.declare_dram_parameter("pts13", [13, PC], BF16, isOutput=False)
    n4_d = nc.declare_dram_parameter("n4", [VT, (V // VT) * 4], F32, isOutput=False)
    out4_d = nc.declare_dram_parameter("out4", [4, PC], F32, isOutput=True)

    n_pb = PC // PB                 # 8 point blocks
    n_vt = V // VT                  # 64 vert tiles
    n_ug = n_vt // UG               # 8 vert-tile groups (unrolled in body)
    n_ch = UG // VG                 # 4 Ln chunks per group

    with ExitStack() as ctx:
        tc = ctx.enter_context(tile.TileContext(nc))
        consts = ctx.enter_context(tc.tile_pool(name="consts", bufs=1))
        psum_r2 = ctx.enter_context(tc.tile_pool(name="psum_r2", bufs=3, space="PSUM"))
        psum_out = ctx.enter_context(tc.tile_pool(name="psum_out", bufs=2, space="PSUM"))

        vl13 = consts.tile([13, V], BF16)
        pts13 = consts.tile([13, PC], BF16)
        n4 = consts.tile([VT, (V // VT) * 4], F32)
        n4b = consts.tile([VT, (V // VT) * 4], BF16)
        outsb = consts.tile([4, PC], F32)
        sqrt_bias = consts.tile([VT, 1], F32)
        pslot = consts.tile([13, 2 * PB], BF16)
        m_bufs = [consts.tile([VT, UG * PB], BF16, name=f"mbuf{b}")
                  for b in range(3)]
        m2 = consts.tile([VT, UG * PB], BF16)
        s_bufs = [consts.tile([VT, UG * PB], BF16, name=f"sbuf{b}")
                  for b in range(3)]
        nc.vector.memset(sqrt_bias[:], B_REG)
        nc.sync.dma_start(out=vl13[:], in_=vl13_d.ap())
        nc.sync.dma_start(out=pts13[:], in_=pts13_d.ap())
        nc.sync.dma_start(out=n4[:], in_=n4_d.ap())
        nc.vector.tensor_copy(n4b[:], n4[:])

        def mm1_group(vg):
            """r2 for (vert group, pb half) -> Ln into u -> Exp into s."""
            ug, b = vg
            for ch in range(n_ch):
                r2 = psum_r2.tile([VT, VG * PB], F32, tag="r2")
                for t in range(VG):
                    vt = ug * UG + ch * VG + t
                    nc.tensor.matmul(
                        r2[:, t * PB:(t + 1) * PB],
                        vl13[:, vt * VT:(vt + 1) * VT],
                        pslot[:, b * PB:(b + 1) * PB],
                        start=True,
                        stop=True,
                    )
                # m = (r2 + B_REG)^-0.5; s = m^3 via two bf16 DVE muls
                par = (2 * ug + b) % 3
                nc.scalar.activation(m_bufs[par][:, ch * VG * PB:
                                                 (ch + 1) * VG * PB],
                                     r2[:],
                                     mybir.ActivationFunctionType
                                     .Abs_reciprocal_sqrt,
                                     bias=sqrt_bias[:])
            nc.vector.tensor_mul(m2[:], m_bufs[par][:], m_bufs[par][:])
            nc.vector.tensor_mul(s_bufs[par][:], m2[:], m_bufs[par][:])

        def mm2_group(vg):
            ug, b = vg
            s = s_bufs[(2 * ug + b) % 3]
            acc = psum_out.tile([4, PB], F32, tag="acc")
            for k in range(UG):
                vt = ug * UG + k
                nc.tensor.matmul(
                    acc[:],
                    n4b[:, vt * 4:(vt + 1) * 4],
                    s[:, k * PB:(k + 1) * PB],
                    start=(k == 0),
                    stop=(k == UG - 1),
                )
            if ug == 0:
                nc.vector.tensor_copy(osls[b], acc[:])
            else:
                nc.vector.tensor_add(osls[b], osls[b], acc[:])

        n_half = n_pb // 2      # two point blocks per iteration
        with tc.For_i(0, reps * n_half) as t_iv:
            # pair of point blocks (reps wrap around, idempotent)
            i2 = (t_iv % n_half) * 2
            nc.vector.tensor_copy(pslot[:], pts13[:, ds(i2 * PB, 2 * PB)])
            osls = [outsb[:, ds((i2 + b) * PB, PB)] for b in range(2)]
            # 16 virtual groups (vert group, point-block half); mm2 of
            # group g issues after mm1 of group g+2 (two-group lookahead),
            # covering the ScalarE->VectorE s-production latency so the PE
            # never stalls waiting for s. m/s buffers are 3-deep for this.
            vgs = [(ug, b) for ug in range(n_ug) for b in range(2)]
            mm1_group(vgs[0])
            mm1_group(vgs[1])
            for g in range(2, len(vgs)):
                mm1_group(vgs[g])
                mm2_group(vgs[g - 2])
            mm2_group(vgs[-2])
            mm2_group(vgs[-1])
        nc.sync.dma_start(out=out4_d.ap(), in_=outsb[:])
    nc.finalize()
    return nc


# ------------------------- host-side numerics --------------------------------
def _preprocess_mesh(verts, faces):
    """Bit-exact replica of the reference's areaic normals: jax fp32 on CPU."""
    import jax
    import jax.numpy as jnp

    with jax.default_device(jax.devices("cpu")[0]):
        v = jnp.asarray(verts, jnp.float32)
        f = jnp.asarray(np.asarray(faces).astype(np.int32))
        fv = v[f]
        A = fv[:, 1] - fv[:, 0]
        Bv = fv[:, 2] - fv[:, 1]
        C = fv[:, 0] - fv[:, 2]

        def corner_angle(u, w):
            c = -jnp.sum(u * w, axis=1) / (
                EPS + jnp.linalg.norm(u, axis=1) * jnp.linalg.norm(w, axis=1))
            return jnp.arccos(jnp.clip(c, -1.0, 1.0))

        angles = jnp.stack(
            [corner_angle(C, A), corner_angle(A, Bv), corner_angle(Bv, C)], axis=1)
        s2 = jnp.sin(2.0 * angles)
        w = s2 / (jnp.sum(s2, axis=-1, keepdims=True) + EPS)
        w = (w[:, [2, 0, 1]] + w[:, [1, 2, 0]]) / 2.0

        fn = jnp.cross(A, Bv)
        areas = 0.5 * jnp.linalg.norm(fn, axis=1)

        nv = v.shape[0]
        idx = f.reshape(-1)
        dual_v = jax.ops.segment_sum((w * areas[:, None]).reshape(-1), idx,
                                     num_segments=nv)
        vn = jax.ops.segment_sum(jnp.repeat(fn, 3, axis=0), idx, num_segments=nv)
        vn = vn / (jnp.linalg.norm(vn, axis=1, keepdims=True) + EPS)
        na = np.asarray(vn * dual_v[:, None])
    d = np.sum(na.astype(np.float64) * np.asarray(verts, np.float64), axis=1)
    return na, d.astype(np.float32)


def _near_pairs(points, verts, rcut):
    """(point, vert) pairs with |p-v| < rcut via grid hashing (pure numpy)."""
    from collections import defaultdict

    pts = points.astype(np.float64)
    vts = verts.astype(np.float64)
    vcell = np.floor(vts / rcut).astype(np.int64)
    vmap = defaultdict(list)
    for j, c in enumerate(map(tuple, vcell)):
        vmap[c].append(j)
    vmap = {k: np.asarray(vs) for k, vs in vmap.items()}
    pcell = np.floor(pts / rcut).astype(np.int64)
    order = np.lexsort((pcell[:, 2], pcell[:, 1], pcell[:, 0]))
    pc_sorted = pcell[order]
    bounds = np.nonzero(np.any(np.diff(pc_sorted, axis=0) != 0, axis=1))[0] + 1
    starts = np.concatenate([[0], bounds])
    ends = np.concatenate([bounds, [len(order)]])
    ip_list, iv_list = [], []
    for s0, e0 in zip(starts, ends):
        pidx = order[s0:e0]
        c = pc_sorted[s0]
        cand = [vmap[k] for k in
                ((c[0] + dx, c[1] + dy, c[2] + dz)
                 for dx in (-1, 0, 1) for dy in (-1, 0, 1) for dz in (-1, 0, 1))
                if k in vmap]
        if not cand:
            continue
        cand = np.concatenate(cand)
        diff = vts[None, cand, :] - pts[pidx, None, :]
        r2 = np.sum(diff * diff, axis=2)
        ii, jj = np.nonzero(r2 < rcut * rcut)
        ip_list.append(pidx[ii])
        iv_list.append(cand[jj])
    if not ip_list:
        return np.zeros(0, np.int64), np.zeros(0, np.int64)
    return np.concatenate(ip_list), np.concatenate(iv_list)


def _split_bf16(x32):
    """x32 (fp32) -> (hi, lo) bf16 pair with hi + lo ~= x32."""
    import ml_dtypes

    bf = ml_dtypes.bfloat16
    hi = x32.astype(bf)
    lo = (x32 - hi.astype(np.float32)).astype(bf)
    return hi, lo


def _host_correction(points32, verts32, na, d32, pp32, vv32):
    """wf_corr[p] = sum_near [true_term - device_term_pred] / 4pi.

    The device computes r2 from 13 bf16 hi/lo channel products (exact
    products, fp32 PSUM accumulation), s = cube(bf16((r2+B_REG)^-0.5))
    via the abs_reciprocal_sqrt table and bf16 VectorE multiplies, and
    contracts against bf16 (na|d) weights. All bf16 roundings are
    replicated here exactly (round-to-nearest-even via ml_dtypes); only the
    device's fp32 accumulation order (~1e-6 abs on r2) and the fp32
    activation rounding are approximated by fp64."""
    import ml_dtypes

    bf = ml_dtypes.bfloat16
    ip, iv = _near_pairs(points32, verts32, RCUT)
    p = points32.astype(np.float64)[ip]
    v = verts32.astype(np.float64)[iv]
    diff = v - p
    r2t = np.sum(diff * diff, axis=1)
    s_true = 1.0 / (r2t ** 1.5 + EPS)
    g_true = np.sum(na.astype(np.float64)[iv] * diff, axis=1)
    # replicate the device's 13-channel split-bf16 Gram r2
    vhi, vlo = _split_bf16(verts32)
    q32 = (np.float32(-2.0) * points32).astype(np.float32)
    qhi, qlo = _split_bf16(q32)
    vvhi, vvlo = _split_bf16(vv32)
    pphi, pplo = _split_bf16(pp32)
    vh = vhi.astype(np.float64)[iv]
    vl_ = vlo.astype(np.float64)[iv]
    qh = qhi.astype(np.float64)[ip]
    ql = qlo.astype(np.float64)[ip]
    r2d = (np.sum(vh * qh + vh * ql + vl_ * qh, axis=1)
           + vvhi.astype(np.float64)[iv] + vvlo.astype(np.float64)[iv]
           + pphi.astype(np.float64)[ip] + pplo.astype(np.float64)[ip])
    r2d = np.maximum(r2d, 0.0)
    m_bf = ((r2d + B_REG) ** -0.5).astype(np.float32).astype(bf).astype(np.float64)
    m2_bf = (m_bf * m_bf).astype(bf).astype(np.float64)
    s_dev = (m2_bf * m_bf).astype(bf).astype(np.float64)
    na_bf = na.astype(np.float32).astype(bf).astype(np.float64)
    d_bf = d32.astype(bf).astype(np.float64)
    g_dev = d_bf[iv] - np.sum(p * na_bf[iv], axis=1)
    corr = (s_true * g_true - s_dev * g_dev) / FOUR_PI
    return np.bincount(ip, weights=corr, minlength=points32.shape[0])


# ------------------------------- entry point ---------------------------------
def _prepare(verts, points, faces):
    verts32 = np.ascontiguousarray(np.asarray(verts, np.float32))
    points32 = np.ascontiguousarray(np.asarray(points, np.float32))

    na, d = _preprocess_mesh(verts32, faces)

    import ml_dtypes

    bf = ml_dtypes.bfloat16
    vv32 = np.sum(verts32.astype(np.float64) ** 2, axis=1).astype(np.float32)
    pp32 = np.sum(points32.astype(np.float64) ** 2, axis=1).astype(np.float32)

    vhi, vlo = _split_bf16(verts32)
    q32 = (np.float32(-2.0) * points32).astype(np.float32)
    qhi, qlo = _split_bf16(q32)
    vvhi, vvlo = _split_bf16(vv32)
    pphi, pplo = _split_bf16(pp32)

    vl13 = np.zeros((13, V), bf)
    vl13[0:3] = vhi.T
    vl13[3:6] = vhi.T
    vl13[6:9] = vlo.T
    vl13[9] = vvhi
    vl13[10] = vvlo
    vl13[11] = 1.0
    vl13[12] = 1.0

    # n4: lhsT tiles for mm2 — n4[vp, vt*4+j] = [na | d][vt*128+vp, j]
    nmat = np.concatenate([na.astype(np.float32), d[:, None]], axis=1)  # (V,4)
    n4 = np.ascontiguousarray(
        nmat.reshape(V // VT, VT, 4).transpose(1, 0, 2).reshape(VT, (V // VT) * 4))

    in_maps = []
    for c in range(N_CORES):
        sl = slice(c * PC, (c + 1) * PC)
        pts13 = np.zeros((13, PC), bf)
        pts13[0:3] = qhi[sl].T
        pts13[3:6] = qlo[sl].T
        pts13[6:9] = qhi[sl].T
        pts13[9] = 1.0
        pts13[10] = 1.0
        pts13[11] = pphi[sl]
        pts13[12] = pplo[sl]
        in_maps.append({"vl13": vl13, "pts13": pts13, "n4": n4})
    return in_maps, verts32, points32, (na, d), pp32, vv32


def _finish(core_outs, verts32, points32, nad, pp32, vv32):
    """core_outs: list of (4, PC) arrays. Combine + near-pair correction."""
    wf = np.empty(P, np.float64)
    for c in range(N_CORES):
        sl = slice(c * PC, (c + 1) * PC)
        o = np.asarray(core_outs[c], np.float64)
        pd = points32[sl].astype(np.float64)
        wf[sl] = (o[3] - pd[:, 0] * o[0] - pd[:, 1] * o[1] - pd[:, 2] * o[2]) / FOUR_PI
    na, d32 = nad
    wf += _host_correction(points32, verts32, na, d32, pp32, vv32)
    return wf.astype(np.float32)


def kernel(verts, points, faces):
    import time

    in_maps, verts32, points32, na, pp32, vv32 = _prepare(verts, points, faces)
    last_err = None
    for attempt in range(3):
        try:
            if "nc" not in _NC_CACHE:
                _NC_CACHE["nc"] = _build_nc()
            res = run_bass_kernel_spmd(_NC_CACHE["nc"], in_maps,
                                       list(range(N_CORES)))
            core_outs = [np.asarray(res.results[c]["out4"])
                         for c in range(N_CORES)]
            break
        except Exception as e:  # transient axon/NRT faults: rebuild + retry
            last_err = e
            _NC_CACHE.clear()
            time.sleep(5 * (attempt + 1))
    else:
        raise last_err
    return _finish(core_outs, verts32, points32, na, pp32, vv32)
